# revision 8
# baseline (speedup 1.0000x reference)
"""ALiBi attention (B=4, S=1024, D=1024, H=16, Dk=64) on 8 TRN2 NeuronCores.

Sharding: tensor-parallel over heads — 2 heads per core, all 4 batches.
Per core (SPMD, no collectives):
  - QKV projections with head-sharded weights (Q pre-scaled by 1/sqrt(Dk))
  - scores computed transposed: S^T[k,q] = K_h @ Q_h^T  (PSUM)
  - + ALiBi bias matrix (precomputed on host, symmetric in |i-j|)  (DVE)
  - E = exp(.)  (ACT)
  - U^T[j,q] = [V_h | 1]^T @ E  -> ctx^T and row-sums in one accumulated matmul
  - rr = 1/rowsum broadcast across partitions via K=1 ones-matmul
  - attn^T = E * rr  (DVE) -> DMA out; ctx^T = U^T * rr
  - partial out = ctx^T.T @ Wo_shard -> host sums the 8 partials + bo
Host gathers: attn[b,h,q,k] = attnT[core,h',b,k,q].T ; out = sum(partials) + bo.
"""

import contextlib
import ctypes
import os
import sys
import types

import numpy as np

import concourse.bass as bass
import concourse.mybir as mybir
import concourse.tile as tile
from concourse import bacc
from concourse import bass_utils


def _install_ntff_hook():
    """Provide antenv.axon_hooks if the image lacks it, so
    run_bass_kernel_spmd(trace=True) can capture NTFF profiles."""
    try:
        from antenv.axon_hooks import get_axon_ntff_profile_hook  # noqa
        return
    except ImportError:
        pass
    so_path = "/opt/axon/libaxon_pjrt.so"
    hook = None
    if os.path.exists(so_path):
        lib = ctypes.CDLL(so_path)
        if hasattr(lib, "axon_start_nrt_profile"):
            lib.axon_start_nrt_profile.argtypes = [
                ctypes.POINTER(ctypes.c_int64), ctypes.c_size_t]
            lib.axon_start_nrt_profile.restype = ctypes.c_int64
            lib.axon_stop_nrt_profile.argtypes = [ctypes.c_char_p]
            lib.axon_stop_nrt_profile.restype = ctypes.c_int64

            @contextlib.contextmanager
            def _hook(output_dir, device_ids):
                import jax
                jax.devices()
                if device_ids:
                    ids = (ctypes.c_int64 * len(device_ids))(*device_ids)
                    rc = lib.axon_start_nrt_profile(ids, len(device_ids))
                else:
                    rc = lib.axon_start_nrt_profile(None, 0)
                if rc != 0:
                    raise RuntimeError(f"axon_start_nrt_profile rc={rc}")
                try:
                    yield
                finally:
                    n = lib.axon_stop_nrt_profile(str(output_dir).encode())
                    print(f"profile: {n} file(s) written to {output_dir}",
                          file=sys.stderr)

            hook = _hook
    mod = types.ModuleType("antenv.axon_hooks")
    mod._hook = hook
    mod.get_axon_ntff_profile_hook = lambda: mod._hook
    mod.set_axon_ntff_profile_hook = lambda h: setattr(mod, "_hook", h)
    sys.modules["antenv.axon_hooks"] = mod


_install_ntff_hook()

D_MODEL = 1024
N_HEADS = 16
D_K = 64
B = 4
S = 1024
N_CORES = 8
HPC = N_HEADS // N_CORES          # heads per core = 2
EPC = HPC * D_K                   # head-dim cols per core = 128
BS = B * S                        # 4096
F32 = mybir.dt.float32
F32R = mybir.dt.float32r


def _build_nc():
    nc = bacc.Bacc("TRN2", target_bir_lowering=False, debug=False,
                   num_devices=N_CORES)

    # ---- DRAM I/O (per-core shards; same graph on all 8 cores) ----
    xT = nc.dram_tensor("xT", [D_MODEL, BS], F32R, kind="ExternalInput").ap()
    wqT = nc.dram_tensor("wqT", [D_MODEL, EPC], F32R, kind="ExternalInput").ap()
    wkT = nc.dram_tensor("wkT", [D_MODEL, EPC], F32R, kind="ExternalInput").ap()
    wvT = nc.dram_tensor("wvT", [D_MODEL, EPC], F32R, kind="ExternalInput").ap()
    woS = nc.dram_tensor("woS", [EPC, D_MODEL], F32R, kind="ExternalInput").ap()
    bq = nc.dram_tensor("bq", [EPC, 1], F32, kind="ExternalInput").ap()
    bk = nc.dram_tensor("bk", [EPC, 1], F32, kind="ExternalInput").ap()
    bv = nc.dram_tensor("bv", [EPC, 1], F32, kind="ExternalInput").ap()
    # ALiBi bias matrices for this core's 2 heads: -slope_h * |i-j|
    biasm = nc.dram_tensor("biasm", [HPC, S, S], F32, kind="ExternalInput").ap()

    attnT = nc.dram_tensor("attnT", [HPC, B, S, S], F32,
                           kind="ExternalOutput").ap()
    outp = nc.dram_tensor("outp", [BS, D_MODEL], F32,
                          kind="ExternalOutput").ap()

    KC = S // 128                 # 8 k chunks per (b,h)
    SC = BS // 128                # 32 s chunks globally
    MC = D_MODEL // 128           # 8 contraction chunks for projections

    with tile.TileContext(nc) as tc:
        with (
            tc.tile_pool(name="persist", bufs=1) as pp,
            tc.tile_pool(name="wpool", bufs=1) as wp,
        ):
            # persistent SBUF tensors
            qd = pp.tile([128, BS], F32R, tag="qd")     # Q^T/8  [e, s]
            kd = pp.tile([128, BS], F32R, tag="kd")     # K^T    [e, s]
            vd = pp.tile([128, BS], F32, tag="vd")     # V^T    [e, s]
            # [V_h | 1] tiles: per (h, global s-chunk): [128 k, 65]
            vp = pp.tile([128, HPC * SC * 65], F32R, tag="vp")
            ctxT = pp.tile([128, BS], F32R, tag="ctxT")  # ctx^T [d, s]
            bias_sb = pp.tile([128, KC * S], F32, tag="bias")  # one head
            ones1 = pp.tile([1, 128], F32R, tag="ones1")
            ones1f = pp.tile([1, 128], F32, tag="ones1f")
            onescol = pp.tile([128, 1], F32, tag="onescol")
            nc.gpsimd.memset(ones1f[:, :], 1.0)
            nc.gpsimd.memset(onescol[:, :], 1.0)
            nc.vector.tensor_copy(ones1[:, :], ones1f[:, :])

            ident = pp.tile([128, 128], F32, tag="ident")
            from concourse.masks import make_identity
            make_identity(nc, ident[:, :])

            wq_sb = wp.tile([128, D_MODEL], F32R, tag="wq")
            wk_sb = wp.tile([128, D_MODEL], F32R, tag="wk")
            wv_sb = wp.tile([128, D_MODEL], F32R, tag="wv")
            wo_sb = wp.tile([128, D_MODEL], F32R, tag="wo")
            bq_sb = wp.tile([128, 1], F32, tag="bq")
            bk_sb = wp.tile([128, 1], F32, tag="bk")
            bv_sb = wp.tile([128, 1], F32, tag="bv")
            for mc in range(MC):
                nc.sync.dma_start(wq_sb[:, mc * 128:(mc + 1) * 128],
                                  wqT[mc * 128:(mc + 1) * 128, :])
                nc.sync.dma_start(wk_sb[:, mc * 128:(mc + 1) * 128],
                                  wkT[mc * 128:(mc + 1) * 128, :])
                nc.sync.dma_start(wv_sb[:, mc * 128:(mc + 1) * 128],
                                  wvT[mc * 128:(mc + 1) * 128, :])
            nc.sync.dma_start(wo_sb[:, :], woS[:, :])
            nc.sync.dma_start(bq_sb[:, :], bq[:, :])
            nc.sync.dma_start(bk_sb[:, :], bk[:, :])
            nc.sync.dma_start(bv_sb[:, :], bv[:, :])

            # ---------- Phase A: QKV projections ----------
            with (
                tc.tile_pool(name="xa", bufs=3) as xa,
                tc.tile_pool(name="pa", bufs=2, space="PSUM") as pa,
            ):
                for sc2 in range(BS // 512):          # 8 chunks of 512
                    psq = pa.tile([128, 512], F32, tag="psq")
                    psk = pa.tile([128, 512], F32, tag="psk")
                    psv = pa.tile([128, 512], F32, tag="psv")
                    for mc in range(MC):
                        xt = xa.tile([128, 512], F32R, tag="xt")
                        nc.sync.dma_start(
                            xt[:, :],
                            xT[mc * 128:(mc + 1) * 128,
                               sc2 * 512:(sc2 + 1) * 512])
                        st = (mc == 0)
                        sp = (mc == MC - 1)
                        for ps, w in ((psq, wq_sb), (psk, wk_sb), (psv, wv_sb)):
                            nc.tensor.matmul(
                                ps[:, :],
                                w[:, mc * 128:(mc + 1) * 128],
                                xt[:, :],
                                start=st, stop=sp)
                    sl = (slice(None), slice(sc2 * 512, (sc2 + 1) * 512))
                    nc.vector.tensor_scalar_add(qd[sl], psq[:, :], bq_sb[:, :])
                    nc.vector.tensor_scalar_add(kd[sl], psk[:, :], bk_sb[:, :])
                    nc.vector.tensor_scalar_add(vd[sl], psv[:, :], bv_sb[:, :])

            # ---------- Phase A2: V^T tiles with ones column ----------
            with (
                tc.tile_pool(name="vt", bufs=2, space="PSUM") as vtp,
            ):
                for sc in range(SC):
                    vt_ps = vtp.tile([128, 128], F32, tag="vt")
                    nc.tensor.transpose(vt_ps[:, :],
                                        vd[:, sc * 128:(sc + 1) * 128],
                                        ident[:, :])
                    for h in range(HPC):
                        off = (h * SC + sc) * 65
                        nc.vector.tensor_copy(vp[:, off:off + 64],
                                              vt_ps[:, h * 64:h * 64 + 64])
                        nc.vector.tensor_copy(vp[:, off + 64:off + 65],
                                              onescol[:, :])

            # ---------- Phase B: attention ----------
            with (
                tc.tile_pool(name="stp", bufs=2, space="PSUM") as stp,
                tc.tile_pool(name="utp", bufs=1, space="PSUM") as utp,
                tc.tile_pool(name="rbp", bufs=1, space="PSUM") as rbp,
                tc.tile_pool(name="ep", bufs=10) as ep,
                tc.tile_pool(name="ptp", bufs=3) as ptp,
                tc.tile_pool(name="rrp", bufs=2) as rrp,
                tc.tile_pool(name="rbs", bufs=2) as rbs,
            ):
                for h in range(HPC):
                    hb = h * 64
                    for kc in range(KC):
                        nc.sync.dma_start(
                            bias_sb[:, kc * S:(kc + 1) * S],
                            biasm[h, kc * 128:(kc + 1) * 128, :])
                    for b in range(B):
                        sb = b * S
                        e_tiles = []
                        for kc in range(KC):
                            st_ps = stp.tile([128, S], F32, tag="st")
                            for qh in range(2):
                                nc.tensor.matmul(
                                    st_ps[:, qh * 512:(qh + 1) * 512],
                                    kd[hb:hb + 64,
                                       sb + kc * 128:sb + (kc + 1) * 128
                                       ],
                                    qd[hb:hb + 64,
                                       sb + qh * 512:sb + (qh + 1) * 512
                                       ],
                                    start=True, stop=True)
                            nc.vector.tensor_tensor(
                                st_ps[:, :], st_ps[:, :],
                                bias_sb[:, kc * S:(kc + 1) * S],
                                mybir.AluOpType.add)
                            et = ep.tile([128, S], F32R, tag="et")
                            nc.scalar.activation(
                                et[:, :], st_ps[:, :],
                                mybir.ActivationFunctionType.Exp)
                            e_tiles.append(et)
                        # U^T = [V|1]^T @ E : [65, S] accumulated over kc
                        ut_ps = utp.tile([65, S], F32, tag="ut")
                        for kc in range(KC):
                            voff = (h * SC + b * KC + kc) * 65
                            for qh in range(2):
                                nc.tensor.matmul(
                                    ut_ps[:, qh * 512:(qh + 1) * 512],
                                    vp[:, voff:voff + 65],
                                    e_tiles[kc][:, qh * 512:(qh + 1) * 512
                                                ],
                                    start=(kc == 0), stop=(kc == KC - 1))
                        # rr = 1/rowsum as a [1, S] row
                        rr = rrp.tile([1, S], F32R, tag="rr")
                        with nc.allow_low_precision(
                                reason="f32r rounding of 1/rowsum is ~tf32; "
                                       "fine at rel_err 2e-2"):
                            nc.vector.reciprocal(rr[:, :], ut_ps[64:65, :])
                        # broadcast rr to 128 partitions via K=1 matmul
                        rrb_ps = rbp.tile([128, S], F32, tag="rrb")
                        for qh in range(2):
                            nc.tensor.matmul(
                                rrb_ps[:, qh * 512:(qh + 1) * 512],
                                ones1[:, :],
                                rr[:, qh * 512:(qh + 1) * 512],
                                start=True, stop=True)
                        rrb_sb = rbs.tile([128, S], F32, tag="rrbs")
                        nc.vector.tensor_copy(rrb_sb[:, :], rrb_ps[:, :])
                        # ctx^T[d, s] = U^T[0:64] * rr  (row broadcast)
                        nc.vector.tensor_tensor(
                            ctxT[hb:hb + 64, sb:sb + S],
                            ut_ps[0:64, :], rrb_sb[0:64, :],
                            mybir.AluOpType.mult)
                        # attn^T tiles = E * rr -> DMA out
                        for kc in range(KC):
                            pt = ptp.tile([128, S], F32, tag="pt")
                            nc.vector.tensor_tensor(
                                pt[:, :], e_tiles[kc][:, :], rrb_sb[:, :],
                                mybir.AluOpType.mult)
                            nc.sync.dma_start(
                                attnT[h, b, kc * 128:(kc + 1) * 128, :],
                                pt[:, :])

            # ---------- Phase C: output projection (partial) ----------
            with (
                tc.tile_pool(name="op", bufs=2, space="PSUM") as op,
                tc.tile_pool(name="ob", bufs=3) as ob,
            ):
                for sc in range(SC):
                    for eh in range(2):
                        o_ps = op.tile([128, 512], F32, tag="o")
                        nc.tensor.matmul(
                            o_ps[:, :],
                            ctxT[:, sc * 128:(sc + 1) * 128],
                            wo_sb[:, eh * 512:(eh + 1) * 512],
                            start=True, stop=True)
                        o_sb = ob.tile([128, 512], F32, tag="osb")
                        nc.scalar.copy(o_sb[:, :], o_ps[:, :])
                        nc.sync.dma_start(
                            outp[sc * 128:(sc + 1) * 128,
                                 eh * 512:(eh + 1) * 512],
                            o_sb[:, :])

    nc.compile()
    return nc


_NC_CACHE = None


def _get_nc():
    global _NC_CACHE
    if _NC_CACHE is None:
        _NC_CACHE = _build_nc()
    return _NC_CACHE


def kernel(x, Wq, bq, Wk, bk, Wv, bv, Wo, bo):
    x = np.asarray(x, np.float32)
    Wq = np.asarray(Wq, np.float32); bq = np.asarray(bq, np.float32)
    Wk = np.asarray(Wk, np.float32); bk = np.asarray(bk, np.float32)
    Wv = np.asarray(Wv, np.float32); bv = np.asarray(bv, np.float32)
    Wo = np.asarray(Wo, np.float32); bo = np.asarray(bo, np.float32)

    nc = _get_nc()

    xT = np.ascontiguousarray(x.reshape(BS, D_MODEL).T)     # [D, BS]
    scale = 1.0 / np.sqrt(np.float32(D_K))
    # ALiBi slopes
    slopes = 2.0 ** (-8.0 * (np.arange(1, N_HEADS + 1, dtype=np.float32)
                             / N_HEADS))
    pos = np.arange(S, dtype=np.float32)
    dist = np.abs(pos[:, None] - pos[None, :])               # [S, S]

    in_maps = []
    for c in range(N_CORES):
        cols = slice(c * EPC, (c + 1) * EPC)                 # head cols
        m = {
            "xT": xT,
            "wqT": np.ascontiguousarray(Wq.T[:, cols]) * scale,
            "wkT": np.ascontiguousarray(Wk.T[:, cols]),
            "wvT": np.ascontiguousarray(Wv.T[:, cols]),
            "woS": np.ascontiguousarray(Wo.T[cols, :]),
            "bq": (bq[cols] * scale).reshape(EPC, 1),
            "bk": bk[cols].reshape(EPC, 1),
            "bv": bv[cols].reshape(EPC, 1),
            "biasm": np.stack([
                -slopes[c * HPC + h] * dist for h in range(HPC)]),
        }
        in_maps.append({k: np.ascontiguousarray(v, np.float32)
                        for k, v in m.items()})

    want_trace = bool(int(os.environ.get("ATTN_TRACE", "0")))
    try:
        res = bass_utils.run_bass_kernel_spmd(
            nc, in_maps, core_ids=list(range(N_CORES)),
            trace=want_trace,
            tmpdir="/tmp/attn_trace" if want_trace else None)
    except Exception:
        if not want_trace:
            raise
        import traceback
        traceback.print_exc()
        res = bass_utils.run_bass_kernel_spmd(
            nc, in_maps, core_ids=list(range(N_CORES)), trace=False)
    if res.exec_time_ns is not None:
        print(f"HW exec time: {res.exec_time_ns} ns")
        kernel.last_exec_time_ns = res.exec_time_ns
        kernel.last_trace = getattr(res, "profile_json", None)

    out = np.zeros((BS, D_MODEL), np.float32)
    attn = np.empty((B, N_HEADS, S, S), np.float32)
    for c in range(N_CORES):
        r = res.results[c]
        out += r["outp"]
        at = r["attnT"]                                      # [HPC,B,S,S] k,q
        for h in range(HPC):
            attn[:, c * HPC + h] = at[h].transpose(0, 2, 1)
    out = (out + bo).reshape(B, S, D_MODEL)
    return out, attn
